# revision 9
# baseline (speedup 1.0000x reference)
"""CRF message-passing kernel for 8 Trainium2 NeuronCores.

node_pot = x @ W_node.T + b_node                          [N, 2]
edge_pot = x[eu] @ Wu.T + x[ev] @ Wv.T + b_edge           [E, 4]
  (Wu = W_edge[:, :F], Wv = W_edge[:, F:])

Per NeuronCore (edges sharded 8 ways, x replicated):
  Phase 1: weights-stationary matmuls compute per-node projections
    A_c = x@Wu[c].T, B_c = x@Wv[c].T for all nodes as column tables in
    SBUF. The node range is split into 4 quarters; four replicated
    [128,128] weight matrices accumulate into one PSUM chunk so each
    SBUF partition receives the quarter-column its Q7 core's bucket
    needs. Edge bucket type t=(u_quarter, v_quarter) of 16 types; Q7
    core k serves two types (lanes 0-7 and 8-15).
  Phase 2: GPSIMD indirect_copy gathers the tables with per-core
    4-phase index streams [uA, vA, uB, vB] (uint16, quarter-relative).
    A one-hot matmul shifts partitions by +4; DVE adds A[u]+B[v].
    Strided-partition stores write 8 column planes; host reassembles.
  node_pot: exact-f32 node-major matmul pass over each core's N-shard.

The gather table lives at a byte offset >= 4*(QTR-1) inside its tile:
indirect_copy's 3-index read pattern issues a garbage 4th read at
base + (i1+i2-i0)*4 which must not underflow partition address 0.
"""

import numpy as np
import ml_dtypes

N, F, E, C = 50000, 128, 800000, 2
NCORES = 8
NP = 53248          # padded node count: 4 * 13312 = 104 * 512
QTR = NP // 4       # 13312
NCHK = 26           # table chunks of 512 per quarter
TPAD = 13440        # dead f32 elems before the table (>= QTR-1, mult of 64)
EC = E // NCORES    # 100000 edges per NeuronCore
NSH = 6250          # node_pot shard per core
NSHP = 6272         # padded: 49 * 128
JN = 49

_CACHE = {}
LAST_RESULTS = None


def _build(ch):
    import concourse.bacc as bacc
    import concourse.tile as tile
    from concourse import mybir

    f32 = mybir.dt.float32
    bf16 = mybir.dt.bfloat16
    u16 = mybir.dt.uint16

    nc = bacc.Bacc(None, target_bir_lowering=False, debug=False)

    xtb = nc.dram_tensor("xtb", [128, NP], bf16, kind="ExternalInput")
    xtn = nc.dram_tensor("xtn", [128, NSHP], f32, kind="ExternalInput")
    wq = nc.dram_tensor("wq", [128, 512], bf16, kind="ExternalInput")
    s4 = nc.dram_tensor("s4", [128, 128], f32, kind="ExternalInput")
    biasv = nc.dram_tensor("biasv", [128, 1], f32, kind="ExternalInput")
    wn = nc.dram_tensor("wn", [128, 2], f32, kind="ExternalInput")
    bnp = nc.dram_tensor("bnp", [128, 2 * JN], f32, kind="ExternalInput")
    eidx = nc.dram_tensor("eidx", [128, ch * 64], u16, kind="ExternalInput")
    npo_d = nc.dram_tensor("npo", [NSHP, 2], f32, kind="ExternalOutput")
    eot_d = nc.dram_tensor("eot", [8, 8, ch * 512], f32, kind="ExternalOutput")

    with tile.TileContext(nc) as tc:
        with (
            tc.tile_pool(name="tabp", bufs=1) as tabp,
            tc.tile_pool(name="cpool", bufs=1) as cpool,
            tc.tile_pool(name="npp", bufs=1) as npp,
            tc.tile_pool(name="xp", bufs=2) as xp,
            tc.tile_pool(name="gp", bufs=3) as gp,
            tc.tile_pool(name="ip", bufs=3) as ip,
            tc.tile_pool(name="rp", bufs=2) as rp,
            tc.tile_pool(name="op", bufs=1) as op,
            tc.tile_pool(name="ps_np", bufs=1, space="PSUM") as ps_np,
            tc.tile_pool(name="ps_tab", bufs=2, space="PSUM") as ps_tab,
            tc.tile_pool(name="ps_sh", bufs=2, space="PSUM") as ps_sh,
        ):
            # table tile first so it lands at the lowest SBUF offset;
            # the live table starts TPAD f32-elements in.
            big = tabp.tile([128, TPAD + QTR], f32)
            tab = big[:, TPAD:TPAD + QTR]

            wq_t = cpool.tile([128, 512], bf16)
            nc.scalar.dma_start(wq_t[:], wq[:])
            s4_t = cpool.tile([128, 128], f32)
            nc.scalar.dma_start(s4_t[:], s4[:])
            bias_t = cpool.tile([128, 1], f32)
            nc.scalar.dma_start(bias_t[:], biasv[:])
            wn_t = cpool.tile([128, 2], f32)
            nc.scalar.dma_start(wn_t[:], wn[:])
            bnp_t = cpool.tile([128, 2 * JN], f32)
            nc.scalar.dma_start(bnp_t[:], bnp[:])

            # ---- node_pot pass (exact f32, node-major), split in 2 ----
            for h, (i0, nmm) in enumerate([(0, 24), (24, 25)]):
                xn_t = npp.tile([128, 25 * 128], f32, tag="xn")
                nc.sync.dma_start(
                    xn_t[:, :128 * nmm],
                    xtn[:, 128 * i0:128 * (i0 + nmm)],
                )
                nps = ps_np.tile([128, 2 * 25], f32, tag="nps")
                for i in range(nmm):
                    nc.tensor.matmul(
                        nps[:, 2 * i:2 * i + 2],
                        lhsT=xn_t[:, 128 * i:128 * (i + 1)],
                        rhs=wn_t[:],
                        start=True,
                        stop=True,
                    )
                npo_t = op.tile([128, 2 * 25], f32, tag="npo")
                nc.vector.tensor_add(
                    out=npo_t[:, :2 * nmm],
                    in0=nps[:, :2 * nmm],
                    in1=bnp_t[:, 2 * i0:2 * (i0 + nmm)],
                )
                nc.sync.dma_start(
                    npo_d.rearrange("(p i) c -> p i c", p=128)[:, i0:i0 + nmm],
                    npo_t[:, :2 * nmm].rearrange("p (i c) -> p i c", c=2),
                )

            # ---- edge table fill ----
            copyf = mybir.ActivationFunctionType.Identity
            ci = 0
            while ci < NCHK:
                g = min(2, NCHK - ci)
                xq_ts = []
                for q in range(4):
                    xq_t = xp.tile([128, 1024], bf16, tag=f"xq{q}")
                    nc.sync.dma_start(
                        xq_t[:, :512 * g],
                        xtb[:, q * QTR + 512 * ci:q * QTR + 512 * (ci + g)],
                    )
                    xq_ts.append(xq_t)
                for j in range(g):
                    ps = ps_tab.tile([128, 512], f32)
                    for q in range(4):
                        nc.tensor.matmul(
                            ps[:],
                            lhsT=wq_t[:, 128 * q:128 * (q + 1)],
                            rhs=xq_ts[q][:, 512 * j:512 * (j + 1)],
                            start=(q == 0),
                            stop=(q == 3),
                        )
                    cc = ci + j
                    nc.scalar.activation(
                        tab[:, 512 * cc:512 * (cc + 1)],
                        ps[:],
                        copyf,
                        bias=bias_t[:, 0:1],
                        scale=1.0,
                    )
                ci += g

            # ---- edge gather + combine ----
            ngroup = (ch + 3) // 4
            for gi in range(ngroup):
                nch = min(4, ch - gi * 4)
                racc = rp.tile([128, 2048], f32, tag="racc")
                for j in range(nch):
                    ci2 = gi * 4 + j
                    it = ip.tile([128, 64], u16)
                    nc.scalar.dma_start(it[:], eidx[:, 64 * ci2:64 * (ci2 + 1)])
                    g2 = gp.tile([128, 1024], f32)
                    nc.gpsimd.indirect_copy(
                        out=g2[:],
                        data=tab,
                        idxs=it[:],
                        i_know_ap_gather_is_preferred=True,
                    )
                    ps2 = ps_sh.tile([128, 512], f32)
                    nc.tensor.matmul(
                        ps2[:],
                        lhsT=s4_t[:],
                        rhs=g2[:].rearrange("p (j two) -> p j two", two=2)[:, :, 1],
                        start=True,
                        stop=True,
                    )
                    nc.vector.tensor_add(
                        out=racc[:, 512 * j:512 * (j + 1)],
                        in0=g2[:].rearrange("p (j two) -> p j two", two=2)[:, :, 0],
                        in1=ps2[:],
                    )
                for ri, lane in enumerate([0, 1, 2, 3, 8, 9, 10, 11]):
                    nc.sync.dma_start(
                        eot_d[ri, :, 2048 * gi:2048 * gi + 512 * nch],
                        racc[lane::16, :512 * nch],
                    )
    nc.compile()
    return nc


def kernel(x, edge_index, W_node, b_node, W_edge, b_edge, _trace=False):
    global LAST_RESULTS
    from concourse.bass_utils import run_bass_kernel_spmd

    x = np.asarray(x, dtype=np.float32)
    ei = np.asarray(edge_index, dtype=np.int64)
    W_node = np.asarray(W_node, dtype=np.float32)
    b_node = np.asarray(b_node, dtype=np.float32)
    W_edge = np.asarray(W_edge, dtype=np.float32)
    b_edge = np.asarray(b_edge, dtype=np.float32)

    xpad = np.zeros((NP, F), np.float32)
    xpad[:N] = x
    xt32 = np.ascontiguousarray(xpad.T)               # [128, NP] f32
    xtb = xt32.astype(ml_dtypes.bfloat16)             # [128, NP] bf16

    s4m = np.zeros((128, 128), np.float32)
    s4m[np.arange(4, 128), np.arange(0, 124)] = 1.0
    wnm = np.ascontiguousarray(W_node.T)              # [128, 2]
    bnpm = np.tile(b_node, (128, JN)).astype(np.float32)

    # ---- per-NeuronCore edge bucketing ----
    in_maps = []
    meta = []
    pairs_by_core = []
    maxlen = 0
    for cid in range(NCORES):
        eu = ei[0, cid * EC:(cid + 1) * EC]
        ev = ei[1, cid * EC:(cid + 1) * EC]
        qa = eu // QTR
        qb = ev // QTR
        ty = qa * 4 + qb
        counts = np.bincount(ty, minlength=16)
        order = np.argsort(-counts, kind="stable")
        # pair largest with smallest; assign pair i to Q7 core i
        pairs = [(int(order[i]), int(order[15 - i])) for i in range(8)]
        pairs_by_core.append(pairs)
        sel_by_type = [np.flatnonzero(ty == t) for t in range(16)]
        core_meta = []
        streams = []
        for k in range(8):
            ta, tb = pairs[k]
            sa, sb = sel_by_type[ta], sel_by_type[tb]
            na, nb = len(sa), len(sb)
            quads = max(na, nb)
            st = np.zeros(4 * quads, np.uint16)
            st[0:4 * na:4] = (eu[sa] - (ta >> 2) * QTR).astype(np.uint16)
            st[1:4 * na:4] = (ev[sa] - (ta & 3) * QTR).astype(np.uint16)
            st[2:4 * nb:4] = (eu[sb] - (tb >> 2) * QTR).astype(np.uint16)
            st[3:4 * nb:4] = (ev[sb] - (tb & 3) * QTR).astype(np.uint16)
            streams.append(st)
            core_meta.append((sa, sb, na, nb))
            maxlen = max(maxlen, len(st))
        meta.append(core_meta)
        in_maps.append({"streams": streams})  # temp; filled below

    ch = max(1, -(-maxlen // 1024))
    nc = _CACHE.get(ch)
    if nc is None:
        nc = _build(ch)
        _CACHE[ch] = nc

    for cid in range(NCORES):
        streams = in_maps[cid].pop("streams")
        eidx = np.zeros((128, ch * 64), np.uint16)
        for k in range(8):
            st = streams[k]
            buf = np.zeros(ch * 1024, np.uint16)
            buf[:len(st)] = st
            eidx[16 * k:16 * (k + 1), :] = buf.reshape(ch * 64, 16).T

        # replicated per-quarter weights, per this core's type->lane map
        WQ = np.zeros((4, F, 128), np.float32)
        biasv = np.zeros((128, 1), np.float32)
        pairs = pairs_by_core[cid]
        for k in range(8):
            for s, t in enumerate(pairs[k]):
                tqa, tqb = t >> 2, t & 3
                for cl in range(4):
                    WQ[tqa][:, 16 * k + 8 * s + cl] = W_edge[cl, :F]
                    WQ[tqb][:, 16 * k + 8 * s + 4 + cl] = W_edge[cl, F:]
                biasv[16 * k + 8 * s:16 * k + 8 * s + 4, 0] = b_edge
        wqm = np.concatenate(
            [WQ[q] for q in range(4)], axis=1
        ).astype(ml_dtypes.bfloat16)  # [128, 512]

        c0 = cid * NSH
        xtn = np.ascontiguousarray(xt32[:, c0:c0 + NSHP])
        in_maps[cid].update({
            "xtb": xtb, "xtn": xtn, "wq": wqm, "s4": s4m,
            "biasv": biasv, "wn": wnm, "bnp": bnpm, "eidx": eidx,
        })

    kwargs = {}
    if _trace:
        kwargs = dict(trace=True, tmpdir="/tmp/kernel_trace")
    res = run_bass_kernel_spmd(
        nc, in_maps, core_ids=list(range(NCORES)), **kwargs
    )
    LAST_RESULTS = res

    # ---- host-side reassembly ----
    node_pot = np.empty((N, C), np.float32)
    edge_pot = np.empty((E, C * C), np.float32)
    ii = np.arange(NSH)
    rr = (ii % 128) * JN + ii // 128
    for cid in range(NCORES):
        npo = res.results[cid]["npo"]
        node_pot[cid * NSH:(cid + 1) * NSH] = npo[rr]
        eot = res.results[cid]["eot"]  # [8, 8, ch*512]
        for k in range(8):
            sa, sb, na, nb = meta[cid][k]
            if na:
                edge_pot[cid * EC + sa] = eot[0:4, k, 0::2][:, :na].T
            if nb:
                edge_pot[cid * EC + sb] = eot[4:8, k, 1::2][:, :nb].T
    return node_pot, edge_pot
